# revision 10
# baseline (speedup 1.0000x reference)
"""Trainium2 Bass kernel for nn_Actor_67422396612916 (GNN message passing).

Data-parallel over batch B=16 across 8 NeuronCores (2 batches/core).
Per batch (N=1024 nodes, E=4 edge types folded to one adjacency sum):
    adj_s = adj_raw[..., 1:].sum(-1)              (N, N)
    h1 = node @ W1 + b1 ; h1 = adj_s @ h1 + h1    (N, 64)
    h2 = h1 @ W2 + b2   ; h  = adj_s @ h2 + h2    (N, 32)
    x = [h, node] ; gate = sig(x@Ws+bs)*tanh(x@Wt+bt)
    g = tanh(sum_n gate) ; MLP ; out = g @ Wl + bl  (16,)

Device strategy per core:
  - Stream adj rows in 8 tiles of (128, 1024, 5) fp32 (contiguous 2.6MB DMAs).
  - Edge-sum on DVE+GPSIMD, transpose the summed tile on the PE
    (128x128 chunks) into S^T layout (m on partitions) needed for the
    message matmuls; both graph convs reuse the same S^T tiles.
  - conv1 for row-tile i only needs stream tile i -> pipelined with DMA.
  - conv2 + gated aggregation + MLP after the stream; overlaps next batch.
"""

import os
import sys

import numpy as np

if "/opt/trn_rl_repo" not in sys.path:
    sys.path.insert(0, "/opt/trn_rl_repo")

B, N, C = 16, 1024, 32          # batch, nodes, node feat
GC1, GC2 = 64, 32
AUX = 128
Z = 16
NCORES = 8
BPC = B // NCORES               # batches per core
P = 128                         # partition tile
NT = N // P                     # n tiles per batch

_STATE = {}


def _build():
    import concourse.bass as bass
    import concourse.mybir as mybir
    import concourse.tile as tile
    from concourse import bacc
    from contextlib import ExitStack

    f32 = mybir.dt.float32
    AF = mybir.ActivationFunctionType
    OP = mybir.AluOpType

    nc = bacc.Bacc(
        "TRN2",
        target_bir_lowering=False,
        debug=False,
        enable_asserts=False,
        num_devices=NCORES,
    )

    node_e = nc.declare_dram_parameter("node", [BPC, N, C], f32, isOutput=False)
    adj_e = nc.declare_dram_parameter("adj_raw", [BPC, N, N, 5], f32, isOutput=False)
    w1a_e = nc.declare_dram_parameter("W1a", [C + 1, GC1], f32, isOutput=False)
    w2a_e = nc.declare_dram_parameter("W2a", [GC1 + 1, GC2], f32, isOutput=False)
    wsa_e = nc.declare_dram_parameter("Wsa", [GC2 + C + 1, AUX], f32, isOutput=False)
    wta_e = nc.declare_dram_parameter("Wta", [GC2 + C + 1, AUX], f32, isOutput=False)
    wm1_e = nc.declare_dram_parameter("Wm1", [AUX, 128], f32, isOutput=False)
    bm1_e = nc.declare_dram_parameter("bm1c", [128, 1], f32, isOutput=False)
    wm2_e = nc.declare_dram_parameter("Wm2", [128, 128], f32, isOutput=False)
    bm2_e = nc.declare_dram_parameter("bm2c", [128, 1], f32, isOutput=False)
    wl_e = nc.declare_dram_parameter("Wl", [128, Z], f32, isOutput=False)
    bl_e = nc.declare_dram_parameter("blr", [1, Z], f32, isOutput=False)
    id_e = nc.declare_dram_parameter("ident", [P, P], f32, isOutput=False)
    out_e = nc.declare_dram_parameter("out", [BPC, Z], f32, isOutput=True)

    with tile.TileContext(nc) as tc, ExitStack() as ctx:
        const = ctx.enter_context(tc.tile_pool(name="const", bufs=1))
        # persistent weights / identity in SBUF
        w1a = const.tile([C + 1, GC1], f32)
        nc.sync.dma_start(out=w1a[:], in_=w1a_e.ap())
        w2a = const.tile([GC1 + 1, GC2], f32)
        nc.sync.dma_start(out=w2a[:], in_=w2a_e.ap())
        wsa = const.tile([GC2 + C + 1, AUX], f32)
        nc.sync.dma_start(out=wsa[:], in_=wsa_e.ap())
        wta = const.tile([GC2 + C + 1, AUX], f32)
        nc.sync.dma_start(out=wta[:], in_=wta_e.ap())
        wm1 = const.tile([AUX, 128], f32)
        nc.sync.dma_start(out=wm1[:], in_=wm1_e.ap())
        bm1 = const.tile([128, 1], f32)
        nc.sync.dma_start(out=bm1[:], in_=bm1_e.ap())
        wm2 = const.tile([128, 128], f32)
        nc.sync.dma_start(out=wm2[:], in_=wm2_e.ap())
        bm2 = const.tile([128, 1], f32)
        nc.sync.dma_start(out=bm2[:], in_=bm2_e.ap())
        wl = const.tile([128, Z], f32)
        nc.sync.dma_start(out=wl[:], in_=wl_e.ap())
        blr = const.tile([1, Z], f32)
        nc.sync.dma_start(out=blr[:], in_=bl_e.ap())
        ident = const.tile([P, P], f32)
        nc.sync.dma_start(out=ident[:], in_=id_e.ap())

        # pools
        rawp = ctx.enter_context(tc.tile_pool(name="raw", bufs=2))
        uvp = ctx.enter_context(tc.tile_pool(name="uv", bufs=2))
        sp = ctx.enter_context(tc.tile_pool(name="s", bufs=2))
        stp = ctx.enter_context(tc.tile_pool(name="st", bufs=2))
        hp = ctx.enter_context(tc.tile_pool(name="h", bufs=2))
        augp = ctx.enter_context(tc.tile_pool(name="aug", bufs=2))
        gsb = ctx.enter_context(tc.tile_pool(name="gsb", bufs=2))
        smp = ctx.enter_context(tc.tile_pool(name="sm", bufs=2))

        ps_tr = ctx.enter_context(tc.tile_pool(name="ps_tr", bufs=2, space="PSUM"))
        ps_sm = ctx.enter_context(tc.tile_pool(name="ps_sm", bufs=2, space="PSUM"))
        ps_mm = ctx.enter_context(tc.tile_pool(name="ps_mm", bufs=2, space="PSUM"))

        for b in range(BPC):
            # ---------------- small path: node transposes + h1 ----------------
            node_nat = hp.tile([P, NT, C], f32, tag="node_nat")
            nc.sync.dma_start(
                out=node_nat[:],
                in_=node_e.ap()[b].rearrange("(t p) c -> p t c", p=P),
            )
            nta = augp.tile([C + 1, N], f32, tag="nta")       # node^T + ones row
            xta = augp.tile([GC2 + C + 1, N], f32, tag="xta")  # [h^T; node^T; ones]
            h1ta = augp.tile([GC1 + 1, N], f32, tag="h1ta")    # h1'^T + ones row
            nc.vector.memset(nta[C : C + 1, :], 1.0)
            nc.vector.memset(xta[GC2 + C : GC2 + C + 1, :], 1.0)
            nc.vector.memset(h1ta[GC1 : GC1 + 1, :], 1.0)

            for t in range(NT):
                pt_n = ps_sm.tile([C, P], f32, tag="ptsm")
                nc.tensor.transpose(pt_n[:], node_nat[:, t, :], ident[:])
                nc.scalar.copy(nta[0:C, t * P : (t + 1) * P], pt_n[:])

            h1_all = hp.tile([P, NT, GC1], f32, tag="h1_all")
            for t in range(NT):
                ps1 = ps_mm.tile([P, GC1], f32, tag="mm")
                nc.tensor.matmul(ps1[:], lhsT=nta[:, t * P : (t + 1) * P], rhs=w1a[:])
                nc.vector.tensor_copy(h1_all[:, t, :], ps1[:])

            # ---------------- heavy path: stream adj, reduce, transpose, conv1
            st_t = stp.tile([P, NT, N], f32, tag="st")  # S^T chunks: [m_part, j, n]
            h1p_all = hp.tile([P, NT, GC1], f32, tag="h1p_all")
            h2_all = hp.tile([P, NT, GC2], f32, tag="h2_all")
            for i in range(NT):
                raw = rawp.tile([P, N, 5], f32, tag="raw")
                nc.sync.dma_start(out=raw[:], in_=adj_e.ap()[b, i * P : (i + 1) * P])
                u = uvp.tile([P, N], f32, tag="u")
                v = uvp.tile([P, N], f32, tag="v")
                s = sp.tile([P, N], f32, tag="s")
                nc.vector.tensor_add(u[:], raw[:, :, 1], raw[:, :, 2])
                nc.gpsimd.tensor_add(v[:], raw[:, :, 3], raw[:, :, 4])
                nc.vector.tensor_add(s[:], u[:], v[:])
                # transpose 8 (128,128) chunks of s into st_t[:, j, i*P:(i+1)*P]
                for hblk in range(2):
                    pt = ps_tr.tile([P, 4 * P], f32, tag="pt")
                    for q in range(4):
                        j = 4 * hblk + q
                        nc.tensor.transpose(
                            pt[:, q * P : (q + 1) * P],
                            s[:, j * P : (j + 1) * P],
                            ident[:],
                        )
                    nc.scalar.copy(
                        st_t[:, 4 * hblk : 4 * hblk + 4, i * P : (i + 1) * P],
                        pt[:].rearrange("p (j n) -> p j n", j=4),
                    )
                # conv1 message for row tile i: needs only stream tile i
                psm = ps_mm.tile([P, GC1], f32, tag="mm")
                for j in range(NT):
                    nc.tensor.matmul(
                        psm[:],
                        lhsT=st_t[:, j, i * P : (i + 1) * P],
                        rhs=h1_all[:, j, :],
                        start=(j == 0),
                        stop=False,
                    )
                nc.tensor.matmul(
                    psm[:], lhsT=ident[:], rhs=h1_all[:, i, :], start=False, stop=True
                )
                nc.vector.tensor_copy(h1p_all[:, i, :], psm[:])
                # h1'^T chunk into h1ta
                pth = ps_sm.tile([GC1, P], f32, tag="ptsm")
                nc.tensor.transpose(pth[:], h1p_all[:, i, :], ident[:])
                nc.scalar.copy(h1ta[0:GC1, i * P : (i + 1) * P], pth[:])
                # h2 tile i
                ps2 = ps_mm.tile([P, GC2], f32, tag="mm")
                nc.tensor.matmul(ps2[:], lhsT=h1ta[:, i * P : (i + 1) * P], rhs=w2a[:])
                nc.vector.tensor_copy(h2_all[:, i, :], ps2[:])

            # ---------------- conv2 ----------------
            for i in range(NT):
                ps3 = ps_mm.tile([P, GC2], f32, tag="mm")
                for j in range(NT):
                    nc.tensor.matmul(
                        ps3[:],
                        lhsT=st_t[:, j, i * P : (i + 1) * P],
                        rhs=h2_all[:, j, :],
                        start=(j == 0),
                        stop=False,
                    )
                nc.tensor.matmul(
                    ps3[:], lhsT=ident[:], rhs=h2_all[:, i, :], start=False, stop=True
                )
                # build [h | node] (128, 64) then one transpose -> xta rows 0..63
                h_sb = smp.tile([P, GC2 + C], f32, tag="h_sb")
                nc.vector.tensor_copy(h_sb[:, 0:GC2], ps3[:])
                nc.vector.tensor_copy(h_sb[:, GC2 : GC2 + C], node_nat[:, i, :])
                pthx = ps_sm.tile([GC2 + C, P], f32, tag="ptsm")
                nc.tensor.transpose(pthx[:], h_sb[:], ident[:])
                nc.scalar.copy(xta[0 : GC2 + C, i * P : (i + 1) * P], pthx[:])

            # ---------------- gated aggregation ----------------
            rparts = []
            for c in range(2):
                nch = N // 2
                psg = ps_tr.tile([AUX, nch], f32, tag="pt")
                pst = ps_tr.tile([AUX, nch], f32, tag="pt")
                nc.tensor.matmul(psg[:], lhsT=wsa[:], rhs=xta[:, c * nch : (c + 1) * nch])
                nc.tensor.matmul(pst[:], lhsT=wta[:], rhs=xta[:, c * nch : (c + 1) * nch])
                sg = gsb.tile([AUX, nch], f32, tag="sg")
                tg = gsb.tile([AUX, nch], f32, tag="tg")
                nc.scalar.activation(sg[:], psg[:], AF.Sigmoid)
                nc.scalar.activation(tg[:], pst[:], AF.Tanh)
                prod = gsb.tile([AUX, nch], f32, tag="prod")
                r_c = smp.tile([AUX, 1], f32, tag="r_c")
                nc.vector.tensor_mul(prod[:], sg[:], tg[:])
                nc.vector.reduce_sum(r_c[:], prod[:], axis=mybir.AxisListType.X)
                rparts.append(r_c)
            g0 = smp.tile([AUX, 1], f32, tag="g0")
            nc.vector.tensor_add(g0[:], rparts[0][:], rparts[1][:])
            gcol = smp.tile([AUX, 1], f32, tag="gcol")
            nc.scalar.activation(gcol[:], g0[:], AF.Tanh)

            # ---------------- MLP head ----------------
            p1 = ps_mm.tile([128, 1], f32, tag="mm")
            nc.tensor.matmul(p1[:], lhsT=wm1[:], rhs=gcol[:])
            g1 = smp.tile([128, 1], f32, tag="g1")
            nc.scalar.activation(g1[:], p1[:], AF.Tanh, bias=bm1[:])
            p2 = ps_mm.tile([128, 1], f32, tag="mm")
            nc.tensor.matmul(p2[:], lhsT=wm2[:], rhs=g1[:])
            g2 = smp.tile([128, 1], f32, tag="g2")
            nc.scalar.activation(g2[:], p2[:], AF.Tanh, bias=bm2[:])
            p3 = ps_mm.tile([1, Z], f32, tag="mm")
            nc.tensor.matmul(p3[:], lhsT=g2[:], rhs=wl[:])
            ob = smp.tile([1, Z], f32, tag="ob")
            nc.vector.tensor_add(ob[:], p3[:], blr[:])
            nc.sync.dma_start(out=out_e.ap()[b : b + 1, :], in_=ob[:])

    nc.finalize()
    return nc


def _prep_weights(inputs):
    f = np.float32
    W1, b1 = inputs["W1"], inputs["b1"]
    W2, b2 = inputs["W2"], inputs["b2"]
    Ws, bs = inputs["Ws"], inputs["bs"]
    Wt, bt = inputs["Wt"], inputs["bt"]
    Wm1, bm1 = inputs["Wm1"], inputs["bm1"]
    Wm2, bm2 = inputs["Wm2"], inputs["bm2"]
    Wl, bl = inputs["Wl"], inputs["bl"]
    shared = {
        "W1a": np.concatenate([W1, b1[None, :]], 0).astype(f),
        "W2a": np.concatenate([W2, b2[None, :]], 0).astype(f),
        "Wsa": np.concatenate([Ws, bs[None, :]], 0).astype(f),
        "Wta": np.concatenate([Wt, bt[None, :]], 0).astype(f),
        "Wm1": np.ascontiguousarray(Wm1, dtype=f),
        "bm1c": np.ascontiguousarray(bm1.reshape(128, 1), dtype=f),
        "Wm2": np.ascontiguousarray(Wm2, dtype=f),
        "bm2c": np.ascontiguousarray(bm2.reshape(128, 1), dtype=f),
        "Wl": np.ascontiguousarray(Wl, dtype=f),
        "blr": np.ascontiguousarray(bl.reshape(1, Z), dtype=f),
        "ident": np.eye(P, dtype=f),
    }
    return shared


def _ensure_ntff_hook():
    """The agent image's antenv lacks axon_hooks; bass_utils imports it
    unconditionally on the trace path. Shim it and register the ctypes
    NTFF hook against the axon PJRT .so."""
    import types

    try:
        from antenv.axon_hooks import get_axon_ntff_profile_hook  # noqa: F401

        return
    except ImportError:
        pass
    holder = {}
    mod = types.ModuleType("antenv.axon_hooks")
    mod.set_axon_ntff_profile_hook = lambda h: holder.update(h=h)
    mod.get_axon_ntff_profile_hook = lambda: holder.get("h")
    sys.modules["antenv.axon_hooks"] = mod
    import antenv

    antenv.axon_hooks = mod
    so_path = "/opt/axon/libaxon_pjrt.so"
    if os.path.exists(so_path):
        from trn_agent_boot.trn_boot import _ntff_profile_via_ctypes

        mod.set_axon_ntff_profile_hook(_ntff_profile_via_ctypes(so_path))


def kernel(**inputs):
    inputs = {k: np.asarray(v) for k, v in inputs.items()}
    node = inputs["node"].astype(np.float32, copy=False)
    adj = inputs["adj_raw"].astype(np.float32, copy=False)
    shared = _prep_weights(inputs)

    if "nc" not in _STATE:
        _STATE["nc"] = _build()
    nc = _STATE["nc"]

    in_maps = []
    for i in range(NCORES):
        m = dict(shared)
        m["node"] = np.ascontiguousarray(node[i * BPC : (i + 1) * BPC])
        m["adj_raw"] = np.ascontiguousarray(adj[i * BPC : (i + 1) * BPC])
        in_maps.append(m)

    if os.environ.get("KERNEL_SIM") == "1":
        from concourse import bass_interp

        sim = bass_interp.MultiCoreSim(nc, 1)
        for k, vv in in_maps[0].items():
            sim.cores[0].tensor(k)[:] = vv
        sim.simulate()
        out0 = np.array(sim.cores[0].mem_tensor("out"))
        full = np.zeros((B, Z), np.float32)
        full[0:BPC] = out0
        _STATE["exec_ns"] = None
        return full

    from concourse.bass_utils import run_bass_kernel_spmd

    trace = os.environ.get("KERNEL_TRACE") == "1"
    if trace:
        _ensure_ntff_hook()
    tmpdir = os.environ.get("KERNEL_TMPDIR")
    res = run_bass_kernel_spmd(
        nc, in_maps, core_ids=list(range(NCORES)), trace=trace, tmpdir=tmpdir
    )
    _STATE["exec_ns"] = res.exec_time_ns
    _STATE["res"] = res
    out = np.concatenate([res.results[i]["out"] for i in range(NCORES)], 0)
    return out.astype(np.float32)


def last_exec_time_ns():
    return _STATE.get("exec_ns")


# revision 17
# speedup vs baseline: 1.6793x; 1.6793x over previous
"""Trainium2 Bass kernel for nn_Actor_67422396612916 (GNN message passing).

Data-parallel over batch B=16 across 8 NeuronCores (2 batches/core).
Per batch (N=1024 nodes, E=4 edge types folded to one adjacency sum):
    adj_s = adj_raw[..., 1:].sum(-1)              (N, N)
    h1 = node @ W1 + b1 ; h1 = adj_s @ h1 + h1    (N, 64)
    h2 = h1 @ W2 + b2   ; h  = adj_s @ h2 + h2    (N, 32)
    x = [h, node] ; gate = sig(x@Ws+bs)*tanh(x@Wt+bt)
    g = tanh(sum_n gate) ; MLP ; out = g @ Wl + bl  (16,)

v3 design notes:
  - adj streamed fp32 in 8 (128,1024,5) contiguous 2.6MB DMAs per batch;
    edge-sum split DVE+GPSIMD; PE transposes build S^T tiles (fp32)
    reused by both convs.
  - All data stays fp32 in SBUF; matmul operands are float32r views
    (FP22-truncated multiply, ~fp32 accumulate) which run at 1 cy/row
    for moving width >= 256 instead of fp32's 4 cy/row.
  - conv matmuls are reoriented: h tiles are the stationary operand and
    S^T streams as 512-wide moving chunks, so outputs land transposed
    (ready for the next stage) and LDWEIGHTS is 64/32 cols.
  - x is laid out [node; h] (gate weight rows reordered on host) so every
    PSUM->SBUF copy is partition-aligned.
"""

import os
import sys

import numpy as np

if "/opt/trn_rl_repo" not in sys.path:
    sys.path.insert(0, "/opt/trn_rl_repo")

B, N, C = 16, 1024, 32          # batch, nodes, node feat
GC1, GC2 = 64, 32
AUX = 128
Z = 16
NCORES = 8
BPC = B // NCORES               # batches per core
P = 128                         # partition tile
NT = N // P                     # n tiles per batch
NCH = N // 2                    # 512-wide conv chunks

_STATE = {}


def _build():
    import concourse.mybir as mybir
    import concourse.tile as tile
    from concourse import bacc
    from contextlib import ExitStack

    f32 = mybir.dt.float32
    f32r = mybir.dt.float32r
    AF = mybir.ActivationFunctionType

    nc = bacc.Bacc(
        "TRN2",
        target_bir_lowering=False,
        debug=False,
        enable_asserts=False,
        num_devices=NCORES,
    )

    node_e = nc.declare_dram_parameter("node", [BPC, N, C], f32, isOutput=False)
    adj_e = nc.declare_dram_parameter("adj_raw", [BPC, N, N, 5], f32, isOutput=False)
    w1a_e = nc.declare_dram_parameter("W1a", [C + 1, GC1], f32r, isOutput=False)
    w2a_e = nc.declare_dram_parameter("W2a", [GC1 + 1, GC2], f32r, isOutput=False)
    wsna_e = nc.declare_dram_parameter("Wsna", [C + 1, AUX], f32r, isOutput=False)
    wsh_e = nc.declare_dram_parameter("Wsh", [GC2, AUX], f32r, isOutput=False)
    wtna_e = nc.declare_dram_parameter("Wtna", [C + 1, AUX], f32r, isOutput=False)
    wth_e = nc.declare_dram_parameter("Wth", [GC2, AUX], f32r, isOutput=False)
    wm1_e = nc.declare_dram_parameter("Wm1", [AUX, 128], f32, isOutput=False)
    bm1_e = nc.declare_dram_parameter("bm1c", [128, 1], f32, isOutput=False)
    wm2_e = nc.declare_dram_parameter("Wm2", [128, 128], f32, isOutput=False)
    bm2_e = nc.declare_dram_parameter("bm2c", [128, 1], f32, isOutput=False)
    wl_e = nc.declare_dram_parameter("Wl", [128, Z], f32, isOutput=False)
    bl_e = nc.declare_dram_parameter("blr", [1, Z], f32, isOutput=False)
    idf_e = nc.declare_dram_parameter("identf", [P, P], f32, isOutput=False)
    idr_e = nc.declare_dram_parameter("identr", [P, P], f32r, isOutput=False)
    ones_e = nc.declare_dram_parameter("onesr", [1, N], f32r, isOutput=False)
    out_e = nc.declare_dram_parameter("out", [BPC, Z], f32, isOutput=True)

    with tile.TileContext(nc) as tc, ExitStack() as ctx:
        const = ctx.enter_context(tc.tile_pool(name="const", bufs=1))
        w1a = const.tile([C + 1, GC1], f32r)
        nc.sync.dma_start(out=w1a[:], in_=w1a_e.ap())
        w2a = const.tile([GC1 + 1, GC2], f32r)
        nc.sync.dma_start(out=w2a[:], in_=w2a_e.ap())
        wsna = const.tile([C + 1, AUX], f32r)
        nc.sync.dma_start(out=wsna[:], in_=wsna_e.ap())
        wsh = const.tile([GC2, AUX], f32r)
        nc.sync.dma_start(out=wsh[:], in_=wsh_e.ap())
        wtna = const.tile([C + 1, AUX], f32r)
        nc.sync.dma_start(out=wtna[:], in_=wtna_e.ap())
        wth = const.tile([GC2, AUX], f32r)
        nc.sync.dma_start(out=wth[:], in_=wth_e.ap())
        wm1 = const.tile([AUX, 128], f32)
        nc.sync.dma_start(out=wm1[:], in_=wm1_e.ap())
        bm1 = const.tile([128, 1], f32)
        nc.sync.dma_start(out=bm1[:], in_=bm1_e.ap())
        wm2 = const.tile([128, 128], f32)
        nc.sync.dma_start(out=wm2[:], in_=wm2_e.ap())
        bm2 = const.tile([128, 1], f32)
        nc.sync.dma_start(out=bm2[:], in_=bm2_e.ap())
        wl = const.tile([128, Z], f32)
        nc.sync.dma_start(out=wl[:], in_=wl_e.ap())
        blr = const.tile([1, Z], f32)
        nc.sync.dma_start(out=blr[:], in_=bl_e.ap())
        identf = const.tile([P, P], f32)
        nc.sync.dma_start(out=identf[:], in_=idf_e.ap())
        identr = const.tile([P, P], f32r)
        nc.sync.dma_start(out=identr[:], in_=idr_e.ap())

        rawp = ctx.enter_context(tc.tile_pool(name="raw", bufs=2))
        uvp = ctx.enter_context(tc.tile_pool(name="uv", bufs=2))
        sp = ctx.enter_context(tc.tile_pool(name="s", bufs=2))
        stp = ctx.enter_context(tc.tile_pool(name="st", bufs=2))
        hp = ctx.enter_context(tc.tile_pool(name="h", bufs=2))
        augp = ctx.enter_context(tc.tile_pool(name="aug", bufs=2))
        gsb = ctx.enter_context(tc.tile_pool(name="gsb", bufs=1))
        smp = ctx.enter_context(tc.tile_pool(name="sm", bufs=2))

        ps_tr = ctx.enter_context(tc.tile_pool(name="ps_tr", bufs=2, space="PSUM"))
        ps_sm = ctx.enter_context(tc.tile_pool(name="ps_sm", bufs=2, space="PSUM"))
        ps_mm = ctx.enter_context(tc.tile_pool(name="ps_mm", bufs=2, space="PSUM"))

        for b in range(BPC):
            # ---- node: natural tiles + node^T into nta rows 0..31 and
            #      xta rows 0..31 (x layout is [node; h]; weights reordered)
            node_nat = hp.tile([P, NT, C], f32, tag="node_nat")
            nc.sync.dma_start(
                out=node_nat[:],
                in_=node_e.ap()[b].rearrange("(t p) c -> p t c", p=P),
            )
            nta = augp.tile([C + 1, N], f32r, tag="nta")        # [node^T; ones]
            ht = augp.tile([GC2, N], f32r, tag="ht")            # h^T
            h1pta = augp.tile([GC1 + 1, N], f32r, tag="h1pta")  # [h1'^T; ones]
            nc.sync.dma_start(out=nta[C : C + 1, :], in_=ones_e.ap())
            nc.sync.dma_start(out=h1pta[GC1 : GC1 + 1, :], in_=ones_e.ap())

            for t in range(NT):
                pt_n = ps_sm.tile([C, P], f32, tag="ptsm")
                nc.tensor.transpose(pt_n[:], node_nat[:, t, :], identf[:])
                nc.scalar.copy(nta[0:C, t * P : (t + 1) * P], pt_n[:])

            # ---- h1^T = (node @ W1 + b1)^T : (64, 1024)
            h1t = hp.tile([GC1, N], f32r, tag="h1t")
            for cc in range(2):
                psh = ps_mm.tile([GC1, NCH], f32, tag="mm")
                nc.tensor.matmul(
                    psh[:], lhsT=w1a[:], rhs=nta[:, cc * NCH : (cc + 1) * NCH]
                )
                nc.scalar.copy(h1t[:, cc * NCH : (cc + 1) * NCH], psh[:])
            # h1 natural tiles (stationary operand for conv1)
            h1_all = hp.tile([P, NT, GC1], f32r, tag="h1_all")
            for t in range(NT):
                pth = ps_sm.tile([P, GC1], f32r, tag="ptsm")
                nc.tensor.transpose(
                    pth[:], h1t[:, t * P : (t + 1) * P], identr[0:GC1, 0:GC1]
                )
                nc.scalar.copy(h1_all[:, t, :], pth[:])

            # ---- heavy stream: adj tiles -> edge-sum -> S^T
            st_t = stp.tile([P, NT, N], f32r, tag="st")  # S^T: [m_part, j, n]
            for i in range(NT):
                raw = rawp.tile([P, N, 5], f32, tag="raw")
                nc.sync.dma_start(out=raw[:], in_=adj_e.ap()[b, i * P : (i + 1) * P])
                u = uvp.tile([P, N], f32, tag="u")
                v = uvp.tile([P, N], f32, tag="v")
                s = sp.tile([P, N], f32, tag="s")
                nc.vector.tensor_add(u[:], raw[:, :, 1], raw[:, :, 2])
                nc.gpsimd.tensor_add(v[:], raw[:, :, 3], raw[:, :, 4])
                nc.vector.tensor_add(s[:], u[:], v[:])
                for hblk in range(2):
                    pt = ps_tr.tile([P, 4 * P], f32, tag="pt")
                    for q in range(4):
                        j = 4 * hblk + q
                        nc.tensor.transpose(
                            pt[:, q * P : (q + 1) * P],
                            s[:, j * P : (j + 1) * P],
                            identf[:],
                        )
                    nc.scalar.copy(
                        st_t[:, 4 * hblk : 4 * hblk + 4, i * P : (i + 1) * P],
                        pt[:].rearrange("p (j n) -> p j n", j=4),
                    )
                # conv1 on a 512 chunk once its 4 stream tiles are in
                if i % 4 == 3:
                    cc = i // 4
                    ps1 = ps_mm.tile([GC1, NCH], f32, tag="mm")
                    for j in range(NT):
                        nc.tensor.matmul(
                            ps1[:],
                            lhsT=h1_all[:, j, :],
                            rhs=st_t[:, j, cc * NCH : (cc + 1) * NCH],
                            start=(j == 0),
                            stop=False,
                        )
                    nc.tensor.matmul(
                        ps1[:],
                        lhsT=identr[0:GC1, 0:GC1],
                        rhs=h1t[:, cc * NCH : (cc + 1) * NCH],
                        start=False,
                        stop=True,
                    )
                    nc.scalar.copy(h1pta[0:GC1, cc * NCH : (cc + 1) * NCH], ps1[:])

            # ---- h2^T = (h1' @ W2 + b2)^T : (32, 1024)
            h2t = hp.tile([GC2, N], f32r, tag="h2t")
            for cc in range(2):
                psh2 = ps_mm.tile([GC2, NCH], f32, tag="mm")
                nc.tensor.matmul(
                    psh2[:], lhsT=w2a[:], rhs=h1pta[:, cc * NCH : (cc + 1) * NCH]
                )
                nc.scalar.copy(h2t[:, cc * NCH : (cc + 1) * NCH], psh2[:])
            h2_all = hp.tile([P, NT, GC2], f32r, tag="h2_all")
            for t in range(NT):
                pth2 = ps_sm.tile([P, GC2], f32r, tag="ptsm")
                nc.tensor.transpose(
                    pth2[:], h2t[:, t * P : (t + 1) * P], identr[0:GC2, 0:GC2]
                )
                nc.scalar.copy(h2_all[:, t, :], pth2[:])

            # ---- conv2: h^T directly into xta rows 32..63 ----
            for cc in range(2):
                ps2 = ps_mm.tile([GC2, NCH], f32, tag="mm")
                for j in range(NT):
                    nc.tensor.matmul(
                        ps2[:],
                        lhsT=h2_all[:, j, :],
                        rhs=st_t[:, j, cc * NCH : (cc + 1) * NCH],
                        start=(j == 0),
                        stop=False,
                    )
                nc.tensor.matmul(
                    ps2[:],
                    lhsT=identr[0:GC2, 0:GC2],
                    rhs=h2t[:, cc * NCH : (cc + 1) * NCH],
                    start=False,
                    stop=True,
                )
                nc.scalar.copy(ht[:, cc * NCH : (cc + 1) * NCH], ps2[:])

            # ---- gated aggregation ----
            rparts = []
            for cc in range(2):
                psg = ps_tr.tile([AUX, NCH], f32, tag="pt")
                pst = ps_tr.tile([AUX, NCH], f32, tag="pt")
                nc.tensor.matmul(
                    psg[:], lhsT=wsna[:], rhs=nta[:, cc * NCH : (cc + 1) * NCH],
                    start=True, stop=False,
                )
                nc.tensor.matmul(
                    psg[:], lhsT=wsh[:], rhs=ht[:, cc * NCH : (cc + 1) * NCH],
                    start=False, stop=True,
                )
                nc.tensor.matmul(
                    pst[:], lhsT=wtna[:], rhs=nta[:, cc * NCH : (cc + 1) * NCH],
                    start=True, stop=False,
                )
                nc.tensor.matmul(
                    pst[:], lhsT=wth[:], rhs=ht[:, cc * NCH : (cc + 1) * NCH],
                    start=False, stop=True,
                )
                sg = gsb.tile([AUX, NCH], f32, tag="sg")
                tg = gsb.tile([AUX, NCH], f32, tag="tg")
                nc.scalar.activation(sg[:], psg[:], AF.Sigmoid)
                nc.scalar.activation(tg[:], pst[:], AF.Tanh)
                prod = gsb.tile([AUX, NCH], f32, tag="prod")
                r_c = smp.tile([AUX, 1], f32, tag="r_c")
                nc.vector.tensor_mul(prod[:], sg[:], tg[:])
                nc.vector.reduce_sum(r_c[:], prod[:], axis=mybir.AxisListType.X)
                rparts.append(r_c)
            g0 = smp.tile([AUX, 1], f32, tag="g0")
            nc.vector.tensor_add(g0[:], rparts[0][:], rparts[1][:])
            gcol = smp.tile([AUX, 1], f32, tag="gcol")
            nc.scalar.activation(gcol[:], g0[:], AF.Tanh)

            # ---- MLP head (fp32) ----
            p1 = ps_mm.tile([128, 1], f32, tag="mm")
            nc.tensor.matmul(p1[:], lhsT=wm1[:], rhs=gcol[:])
            g1 = smp.tile([128, 1], f32, tag="g1")
            nc.scalar.activation(g1[:], p1[:], AF.Tanh, bias=bm1[:])
            p2 = ps_mm.tile([128, 1], f32, tag="mm")
            nc.tensor.matmul(p2[:], lhsT=wm2[:], rhs=g1[:])
            g2 = smp.tile([128, 1], f32, tag="g2")
            nc.scalar.activation(g2[:], p2[:], AF.Tanh, bias=bm2[:])
            p3 = ps_mm.tile([1, Z], f32, tag="mm")
            nc.tensor.matmul(p3[:], lhsT=g2[:], rhs=wl[:])
            ob = smp.tile([1, Z], f32, tag="ob")
            nc.vector.tensor_add(ob[:], p3[:], blr[:])
            nc.sync.dma_start(out=out_e.ap()[b : b + 1, :], in_=ob[:])

    nc.finalize()
    return nc


def _prep_weights(inputs):
    f = np.float32
    W1, b1 = inputs["W1"], inputs["b1"]
    W2, b2 = inputs["W2"], inputs["b2"]
    Ws, bs = inputs["Ws"], inputs["bs"]
    Wt, bt = inputs["Wt"], inputs["bt"]
    Wm1, bm1 = inputs["Wm1"], inputs["bm1"]
    Wm2, bm2 = inputs["Wm2"], inputs["bm2"]
    Wl, bl = inputs["Wl"], inputs["bl"]
    # gate matmul is split: node-part (rows GC2..GC2+C of Ws) + bias via
    # nta's ones row, then the h-part (rows 0..GC2) against ht.
    shared = {
        "W1a": np.concatenate([W1, b1[None, :]], 0).astype(f),
        "W2a": np.concatenate([W2, b2[None, :]], 0).astype(f),
        "Wsna": np.concatenate([Ws[GC2 : GC2 + C], bs[None, :]], 0).astype(f),
        "Wsh": np.ascontiguousarray(Ws[0:GC2], dtype=f),
        "Wtna": np.concatenate([Wt[GC2 : GC2 + C], bt[None, :]], 0).astype(f),
        "Wth": np.ascontiguousarray(Wt[0:GC2], dtype=f),
        "Wm1": np.ascontiguousarray(Wm1, dtype=f),
        "bm1c": np.ascontiguousarray(bm1.reshape(128, 1), dtype=f),
        "Wm2": np.ascontiguousarray(Wm2, dtype=f),
        "bm2c": np.ascontiguousarray(bm2.reshape(128, 1), dtype=f),
        "Wl": np.ascontiguousarray(Wl, dtype=f),
        "blr": np.ascontiguousarray(bl.reshape(1, Z), dtype=f),
        "identf": np.eye(P, dtype=f),
        "identr": np.eye(P, dtype=f),
        "onesr": np.ones((1, N), f),
    }
    return shared


def _ensure_ntff_hook():
    """The agent image's antenv lacks axon_hooks; bass_utils imports it
    unconditionally on the trace path. Shim it and register the ctypes
    NTFF hook against the axon PJRT .so."""
    import types

    try:
        from antenv.axon_hooks import get_axon_ntff_profile_hook  # noqa: F401

        return
    except ImportError:
        pass
    holder = {}
    mod = types.ModuleType("antenv.axon_hooks")
    mod.set_axon_ntff_profile_hook = lambda h: holder.update(h=h)
    mod.get_axon_ntff_profile_hook = lambda: holder.get("h")
    sys.modules["antenv.axon_hooks"] = mod
    import antenv

    antenv.axon_hooks = mod
    so_path = "/opt/axon/libaxon_pjrt.so"
    if os.path.exists(so_path):
        from trn_agent_boot.trn_boot import _ntff_profile_via_ctypes

        mod.set_axon_ntff_profile_hook(_ntff_profile_via_ctypes(so_path))


def kernel(**inputs):
    inputs = {k: np.asarray(v) for k, v in inputs.items()}
    node = inputs["node"].astype(np.float32, copy=False)
    adj = inputs["adj_raw"].astype(np.float32, copy=False)
    shared = _prep_weights(inputs)

    if "nc" not in _STATE:
        _STATE["nc"] = _build()
    nc = _STATE["nc"]

    in_maps = []
    for i in range(NCORES):
        m = dict(shared)
        m["node"] = np.ascontiguousarray(node[i * BPC : (i + 1) * BPC])
        m["adj_raw"] = np.ascontiguousarray(adj[i * BPC : (i + 1) * BPC])
        in_maps.append(m)

    if os.environ.get("KERNEL_SIM") == "1":
        from concourse import bass_interp

        sim = bass_interp.MultiCoreSim(nc, 1)
        for k, vv in in_maps[0].items():
            sim.cores[0].tensor(k)[:] = vv
        sim.simulate()
        out0 = np.array(sim.cores[0].mem_tensor("out"))
        full = np.zeros((B, Z), np.float32)
        full[0:BPC] = out0
        _STATE["exec_ns"] = None
        return full

    from concourse.bass_utils import run_bass_kernel_spmd

    trace = os.environ.get("KERNEL_TRACE") == "1"
    if trace:
        _ensure_ntff_hook()
    tmpdir = os.environ.get("KERNEL_TMPDIR")
    res = run_bass_kernel_spmd(
        nc, in_maps, core_ids=list(range(NCORES)), trace=trace, tmpdir=tmpdir
    )
    _STATE["exec_ns"] = res.exec_time_ns
    _STATE["res"] = res
    out = np.concatenate([res.results[i]["out"] for i in range(NCORES)], 0)
    return out.astype(np.float32)


def last_exec_time_ns():
    return _STATE.get("exec_ns")


# revision 18
# speedup vs baseline: 1.9099x; 1.1374x over previous
"""Trainium2 Bass kernel for nn_Actor_67422396612916 (GNN message passing).

Data-parallel over batch B=16 across 8 NeuronCores (2 batches/core).
Per batch (N=1024 nodes, E=4 edge types folded to one adjacency sum):
    adj_s = adj_raw[..., 1:].sum(-1)              (N, N)
    h1 = node @ W1 + b1 ; h1 = adj_s @ h1 + h1    (N, 64)
    h2 = h1 @ W2 + b2   ; h  = adj_s @ h2 + h2    (N, 32)
    x = [h, node] ; gate = sig(x@Ws+bs)*tanh(x@Wt+bt)
    g = tanh(sum_n gate) ; MLP ; out = g @ Wl + bl  (16,)

v3 design notes:
  - adj streamed fp32 in 8 (128,1024,5) contiguous 2.6MB DMAs per batch;
    edge-sum split DVE+GPSIMD; PE transposes build S^T tiles (fp32)
    reused by both convs.
  - All data stays fp32 in SBUF; matmul operands are float32r views
    (FP22-truncated multiply, ~fp32 accumulate) which run at 1 cy/row
    for moving width >= 256 instead of fp32's 4 cy/row.
  - conv matmuls are reoriented: h tiles are the stationary operand and
    S^T streams as 512-wide moving chunks, so outputs land transposed
    (ready for the next stage) and LDWEIGHTS is 64/32 cols.
  - x is laid out [node; h] (gate weight rows reordered on host) so every
    PSUM->SBUF copy is partition-aligned.
"""

import os
import sys

import numpy as np

if "/opt/trn_rl_repo" not in sys.path:
    sys.path.insert(0, "/opt/trn_rl_repo")

B, N, C = 16, 1024, 32          # batch, nodes, node feat
GC1, GC2 = 64, 32
AUX = 128
Z = 16
NCORES = 8
BPC = B // NCORES               # batches per core
P = 128                         # partition tile
NT = N // P                     # n tiles per batch
NCH = N // 2                    # 512-wide conv chunks

_STATE = {}


def _build():
    import concourse.mybir as mybir
    import concourse.tile as tile
    from concourse import bacc
    from contextlib import ExitStack

    f32 = mybir.dt.float32
    f32r = mybir.dt.float32r
    AF = mybir.ActivationFunctionType

    nc = bacc.Bacc(
        "TRN2",
        target_bir_lowering=False,
        debug=False,
        enable_asserts=False,
        num_devices=NCORES,
    )

    node_e = nc.declare_dram_parameter("node", [BPC, N, C], f32, isOutput=False)
    adj_e = nc.declare_dram_parameter("adj_raw", [BPC, N, N, 5], f32, isOutput=False)
    w1a_e = nc.declare_dram_parameter("W1a", [C + 1, GC1], f32r, isOutput=False)
    w2a_e = nc.declare_dram_parameter("W2a", [GC1 + 1, GC2], f32r, isOutput=False)
    wsna_e = nc.declare_dram_parameter("Wsna", [C + 1, AUX], f32r, isOutput=False)
    wsh_e = nc.declare_dram_parameter("Wsh", [GC2, AUX], f32r, isOutput=False)
    wtna_e = nc.declare_dram_parameter("Wtna", [C + 1, AUX], f32r, isOutput=False)
    wth_e = nc.declare_dram_parameter("Wth", [GC2, AUX], f32r, isOutput=False)
    wm1_e = nc.declare_dram_parameter("Wm1", [AUX, 128], f32, isOutput=False)
    bm1_e = nc.declare_dram_parameter("bm1c", [128, 1], f32, isOutput=False)
    wm2_e = nc.declare_dram_parameter("Wm2", [128, 128], f32, isOutput=False)
    bm2_e = nc.declare_dram_parameter("bm2c", [128, 1], f32, isOutput=False)
    wl_e = nc.declare_dram_parameter("Wl", [128, Z], f32, isOutput=False)
    bl_e = nc.declare_dram_parameter("blr", [1, Z], f32, isOutput=False)
    idf_e = nc.declare_dram_parameter("identf", [P, P], f32, isOutput=False)
    idr_e = nc.declare_dram_parameter("identr", [P, P], f32r, isOutput=False)
    ones_e = nc.declare_dram_parameter("onesr", [1, N], f32r, isOutput=False)
    out_e = nc.declare_dram_parameter("out", [BPC, Z], f32, isOutput=True)

    with tile.TileContext(nc) as tc, ExitStack() as ctx:
        const = ctx.enter_context(tc.tile_pool(name="const", bufs=1))
        w1a = const.tile([C + 1, GC1], f32r)
        nc.sync.dma_start(out=w1a[:], in_=w1a_e.ap())
        w2a = const.tile([GC1 + 1, GC2], f32r)
        nc.sync.dma_start(out=w2a[:], in_=w2a_e.ap())
        wsna = const.tile([C + 1, AUX], f32r)
        nc.sync.dma_start(out=wsna[:], in_=wsna_e.ap())
        wsh = const.tile([GC2, AUX], f32r)
        nc.sync.dma_start(out=wsh[:], in_=wsh_e.ap())
        wtna = const.tile([C + 1, AUX], f32r)
        nc.sync.dma_start(out=wtna[:], in_=wtna_e.ap())
        wth = const.tile([GC2, AUX], f32r)
        nc.sync.dma_start(out=wth[:], in_=wth_e.ap())
        wm1 = const.tile([AUX, 128], f32)
        nc.sync.dma_start(out=wm1[:], in_=wm1_e.ap())
        bm1 = const.tile([128, 1], f32)
        nc.sync.dma_start(out=bm1[:], in_=bm1_e.ap())
        wm2 = const.tile([128, 128], f32)
        nc.sync.dma_start(out=wm2[:], in_=wm2_e.ap())
        bm2 = const.tile([128, 1], f32)
        nc.sync.dma_start(out=bm2[:], in_=bm2_e.ap())
        wl = const.tile([128, Z], f32)
        nc.sync.dma_start(out=wl[:], in_=wl_e.ap())
        blr = const.tile([1, Z], f32)
        nc.sync.dma_start(out=blr[:], in_=bl_e.ap())
        identf = const.tile([P, P], f32)
        nc.sync.dma_start(out=identf[:], in_=idf_e.ap())
        identr = const.tile([P, P], f32r)
        nc.sync.dma_start(out=identr[:], in_=idr_e.ap())

        rawp = ctx.enter_context(tc.tile_pool(name="raw", bufs=2))
        uvp = ctx.enter_context(tc.tile_pool(name="uv", bufs=2))
        sp = ctx.enter_context(tc.tile_pool(name="s", bufs=2))
        stp = ctx.enter_context(tc.tile_pool(name="st", bufs=2))
        hp = ctx.enter_context(tc.tile_pool(name="h", bufs=2))
        augp = ctx.enter_context(tc.tile_pool(name="aug", bufs=2))
        gsb = ctx.enter_context(tc.tile_pool(name="gsb", bufs=1))
        smp = ctx.enter_context(tc.tile_pool(name="sm", bufs=2))

        ps_tr = ctx.enter_context(tc.tile_pool(name="ps_tr", bufs=2, space="PSUM"))
        ps_sm = ctx.enter_context(tc.tile_pool(name="ps_sm", bufs=2, space="PSUM"))
        ps_mm = ctx.enter_context(tc.tile_pool(name="ps_mm", bufs=2, space="PSUM"))

        for b in range(BPC):
            # ---- node: natural tiles + node^T into nta rows 0..31 and
            #      xta rows 0..31 (x layout is [node; h]; weights reordered)
            node_nat = hp.tile([P, NT, C], f32, tag="node_nat")
            nc.sync.dma_start(
                out=node_nat[:],
                in_=node_e.ap()[b].rearrange("(t p) c -> p t c", p=P),
            )
            nta = augp.tile([C + 1, N], f32r, tag="nta")        # [node^T; ones]
            ht = augp.tile([GC2, N], f32r, tag="ht")            # h^T
            h1pta = augp.tile([GC1 + 1, N], f32r, tag="h1pta")  # [h1'^T; ones]
            nc.sync.dma_start(out=nta[C : C + 1, :], in_=ones_e.ap())
            nc.sync.dma_start(out=h1pta[GC1 : GC1 + 1, :], in_=ones_e.ap())

            for t in range(NT):
                pt_n = ps_sm.tile([C, P], f32, tag="ptsm")
                nc.tensor.transpose(pt_n[:], node_nat[:, t, :], identf[:])
                nc.scalar.copy(nta[0:C, t * P : (t + 1) * P], pt_n[:])

            # ---- h1^T = (node @ W1 + b1)^T : (64, 1024)
            h1t = hp.tile([GC1, N], f32r, tag="h1t")
            for cc in range(2):
                psh = ps_mm.tile([GC1, NCH], f32, tag="mm")
                nc.tensor.matmul(
                    psh[:], lhsT=w1a[:], rhs=nta[:, cc * NCH : (cc + 1) * NCH]
                )
                nc.scalar.copy(h1t[:, cc * NCH : (cc + 1) * NCH], psh[:])
            # h1 natural tiles (stationary operand for conv1)
            h1_all = hp.tile([P, NT, GC1], f32r, tag="h1_all")
            for t in range(NT):
                pth = ps_sm.tile([P, GC1], f32r, tag="ptsm")
                nc.tensor.transpose(
                    pth[:], h1t[:, t * P : (t + 1) * P], identr[0:GC1, 0:GC1]
                )
                nc.scalar.copy(h1_all[:, t, :], pth[:])

            # ---- heavy stream: adj tiles -> edge-sum -> S^T
            st_t = stp.tile([P, NT, N], f32r, tag="st")  # S^T: [m_part, j, n]
            for i in range(NT):
                raw = rawp.tile([P, N, 5], f32, tag="raw")
                eng = nc.sync if i % 2 == 0 else nc.scalar
                eng.dma_start(out=raw[:], in_=adj_e.ap()[b, i * P : (i + 1) * P])
                w = uvp.tile([P, N, 2], f32, tag="w")
                s = sp.tile([P, N], f32, tag="s")
                nc.vector.tensor_add(w[:], raw[:, :, 1:3], raw[:, :, 3:5])
                nc.gpsimd.tensor_add(s[:], w[:, :, 0], w[:, :, 1])
                for hblk in range(2):
                    pt = ps_tr.tile([P, 4 * P], f32, tag="pt")
                    for q in range(4):
                        j = 4 * hblk + q
                        nc.tensor.transpose(
                            pt[:, q * P : (q + 1) * P],
                            s[:, j * P : (j + 1) * P],
                            identf[:],
                        )
                    nc.scalar.copy(
                        st_t[:, 4 * hblk : 4 * hblk + 4, i * P : (i + 1) * P],
                        pt[:].rearrange("p (j n) -> p j n", j=4),
                    )
                # conv1 on a 512 chunk once its 4 stream tiles are in
                if i % 4 == 3:
                    cc = i // 4
                    ps1 = ps_mm.tile([GC1, NCH], f32, tag="mm")
                    for j in range(NT):
                        nc.tensor.matmul(
                            ps1[:],
                            lhsT=h1_all[:, j, :],
                            rhs=st_t[:, j, cc * NCH : (cc + 1) * NCH],
                            start=(j == 0),
                            stop=False,
                        )
                    nc.tensor.matmul(
                        ps1[:],
                        lhsT=identr[0:GC1, 0:GC1],
                        rhs=h1t[:, cc * NCH : (cc + 1) * NCH],
                        start=False,
                        stop=True,
                    )
                    nc.scalar.copy(h1pta[0:GC1, cc * NCH : (cc + 1) * NCH], ps1[:])

            # ---- h2^T = (h1' @ W2 + b2)^T : (32, 1024)
            h2t = hp.tile([GC2, N], f32r, tag="h2t")
            for cc in range(2):
                psh2 = ps_mm.tile([GC2, NCH], f32, tag="mm")
                nc.tensor.matmul(
                    psh2[:], lhsT=w2a[:], rhs=h1pta[:, cc * NCH : (cc + 1) * NCH]
                )
                nc.scalar.copy(h2t[:, cc * NCH : (cc + 1) * NCH], psh2[:])
            h2_all = hp.tile([P, NT, GC2], f32r, tag="h2_all")
            for t in range(NT):
                pth2 = ps_sm.tile([P, GC2], f32r, tag="ptsm")
                nc.tensor.transpose(
                    pth2[:], h2t[:, t * P : (t + 1) * P], identr[0:GC2, 0:GC2]
                )
                nc.scalar.copy(h2_all[:, t, :], pth2[:])

            # ---- conv2: h^T directly into xta rows 32..63 ----
            for cc in range(2):
                ps2 = ps_mm.tile([GC2, NCH], f32, tag="mm")
                for j in range(NT):
                    nc.tensor.matmul(
                        ps2[:],
                        lhsT=h2_all[:, j, :],
                        rhs=st_t[:, j, cc * NCH : (cc + 1) * NCH],
                        start=(j == 0),
                        stop=False,
                    )
                nc.tensor.matmul(
                    ps2[:],
                    lhsT=identr[0:GC2, 0:GC2],
                    rhs=h2t[:, cc * NCH : (cc + 1) * NCH],
                    start=False,
                    stop=True,
                )
                nc.scalar.copy(ht[:, cc * NCH : (cc + 1) * NCH], ps2[:])

            # ---- gated aggregation ----
            rparts = []
            for cc in range(2):
                psg = ps_tr.tile([AUX, NCH], f32, tag="pt")
                pst = ps_tr.tile([AUX, NCH], f32, tag="pt")
                nc.tensor.matmul(
                    psg[:], lhsT=wsna[:], rhs=nta[:, cc * NCH : (cc + 1) * NCH],
                    start=True, stop=False,
                )
                nc.tensor.matmul(
                    psg[:], lhsT=wsh[:], rhs=ht[:, cc * NCH : (cc + 1) * NCH],
                    start=False, stop=True,
                )
                nc.tensor.matmul(
                    pst[:], lhsT=wtna[:], rhs=nta[:, cc * NCH : (cc + 1) * NCH],
                    start=True, stop=False,
                )
                nc.tensor.matmul(
                    pst[:], lhsT=wth[:], rhs=ht[:, cc * NCH : (cc + 1) * NCH],
                    start=False, stop=True,
                )
                sg = gsb.tile([AUX, NCH], f32, tag="sg")
                tg = gsb.tile([AUX, NCH], f32, tag="tg")
                nc.scalar.activation(sg[:], psg[:], AF.Sigmoid)
                nc.scalar.activation(tg[:], pst[:], AF.Tanh)
                prod = gsb.tile([AUX, NCH], f32, tag="prod")
                r_c = smp.tile([AUX, 1], f32, tag="r_c")
                nc.vector.tensor_mul(prod[:], sg[:], tg[:])
                nc.vector.reduce_sum(r_c[:], prod[:], axis=mybir.AxisListType.X)
                rparts.append(r_c)
            g0 = smp.tile([AUX, 1], f32, tag="g0")
            nc.vector.tensor_add(g0[:], rparts[0][:], rparts[1][:])
            gcol = smp.tile([AUX, 1], f32, tag="gcol")
            nc.scalar.activation(gcol[:], g0[:], AF.Tanh)

            # ---- MLP head (fp32) ----
            p1 = ps_mm.tile([128, 1], f32, tag="mm")
            nc.tensor.matmul(p1[:], lhsT=wm1[:], rhs=gcol[:])
            g1 = smp.tile([128, 1], f32, tag="g1")
            nc.scalar.activation(g1[:], p1[:], AF.Tanh, bias=bm1[:])
            p2 = ps_mm.tile([128, 1], f32, tag="mm")
            nc.tensor.matmul(p2[:], lhsT=wm2[:], rhs=g1[:])
            g2 = smp.tile([128, 1], f32, tag="g2")
            nc.scalar.activation(g2[:], p2[:], AF.Tanh, bias=bm2[:])
            p3 = ps_mm.tile([1, Z], f32, tag="mm")
            nc.tensor.matmul(p3[:], lhsT=g2[:], rhs=wl[:])
            ob = smp.tile([1, Z], f32, tag="ob")
            nc.vector.tensor_add(ob[:], p3[:], blr[:])
            nc.gpsimd.dma_start(out=out_e.ap()[b : b + 1, :], in_=ob[:])

    nc.finalize()
    return nc


def _prep_weights(inputs):
    f = np.float32
    W1, b1 = inputs["W1"], inputs["b1"]
    W2, b2 = inputs["W2"], inputs["b2"]
    Ws, bs = inputs["Ws"], inputs["bs"]
    Wt, bt = inputs["Wt"], inputs["bt"]
    Wm1, bm1 = inputs["Wm1"], inputs["bm1"]
    Wm2, bm2 = inputs["Wm2"], inputs["bm2"]
    Wl, bl = inputs["Wl"], inputs["bl"]
    # gate matmul is split: node-part (rows GC2..GC2+C of Ws) + bias via
    # nta's ones row, then the h-part (rows 0..GC2) against ht.
    shared = {
        "W1a": np.concatenate([W1, b1[None, :]], 0).astype(f),
        "W2a": np.concatenate([W2, b2[None, :]], 0).astype(f),
        "Wsna": np.concatenate([Ws[GC2 : GC2 + C], bs[None, :]], 0).astype(f),
        "Wsh": np.ascontiguousarray(Ws[0:GC2], dtype=f),
        "Wtna": np.concatenate([Wt[GC2 : GC2 + C], bt[None, :]], 0).astype(f),
        "Wth": np.ascontiguousarray(Wt[0:GC2], dtype=f),
        "Wm1": np.ascontiguousarray(Wm1, dtype=f),
        "bm1c": np.ascontiguousarray(bm1.reshape(128, 1), dtype=f),
        "Wm2": np.ascontiguousarray(Wm2, dtype=f),
        "bm2c": np.ascontiguousarray(bm2.reshape(128, 1), dtype=f),
        "Wl": np.ascontiguousarray(Wl, dtype=f),
        "blr": np.ascontiguousarray(bl.reshape(1, Z), dtype=f),
        "identf": np.eye(P, dtype=f),
        "identr": np.eye(P, dtype=f),
        "onesr": np.ones((1, N), f),
    }
    return shared


def _ensure_ntff_hook():
    """The agent image's antenv lacks axon_hooks; bass_utils imports it
    unconditionally on the trace path. Shim it and register the ctypes
    NTFF hook against the axon PJRT .so."""
    import types

    try:
        from antenv.axon_hooks import get_axon_ntff_profile_hook  # noqa: F401

        return
    except ImportError:
        pass
    holder = {}
    mod = types.ModuleType("antenv.axon_hooks")
    mod.set_axon_ntff_profile_hook = lambda h: holder.update(h=h)
    mod.get_axon_ntff_profile_hook = lambda: holder.get("h")
    sys.modules["antenv.axon_hooks"] = mod
    import antenv

    antenv.axon_hooks = mod
    so_path = "/opt/axon/libaxon_pjrt.so"
    if os.path.exists(so_path):
        from trn_agent_boot.trn_boot import _ntff_profile_via_ctypes

        mod.set_axon_ntff_profile_hook(_ntff_profile_via_ctypes(so_path))


def kernel(**inputs):
    inputs = {k: np.asarray(v) for k, v in inputs.items()}
    node = inputs["node"].astype(np.float32, copy=False)
    adj = inputs["adj_raw"].astype(np.float32, copy=False)
    shared = _prep_weights(inputs)

    if "nc" not in _STATE:
        _STATE["nc"] = _build()
    nc = _STATE["nc"]

    in_maps = []
    for i in range(NCORES):
        m = dict(shared)
        m["node"] = np.ascontiguousarray(node[i * BPC : (i + 1) * BPC])
        m["adj_raw"] = np.ascontiguousarray(adj[i * BPC : (i + 1) * BPC])
        in_maps.append(m)

    if os.environ.get("KERNEL_SIM") == "1":
        from concourse import bass_interp

        sim = bass_interp.MultiCoreSim(nc, 1)
        for k, vv in in_maps[0].items():
            sim.cores[0].tensor(k)[:] = vv
        sim.simulate()
        out0 = np.array(sim.cores[0].mem_tensor("out"))
        full = np.zeros((B, Z), np.float32)
        full[0:BPC] = out0
        _STATE["exec_ns"] = None
        return full

    from concourse.bass_utils import run_bass_kernel_spmd

    trace = os.environ.get("KERNEL_TRACE") == "1"
    if trace:
        _ensure_ntff_hook()
    tmpdir = os.environ.get("KERNEL_TMPDIR")
    res = run_bass_kernel_spmd(
        nc, in_maps, core_ids=list(range(NCORES)), trace=trace, tmpdir=tmpdir
    )
    _STATE["exec_ns"] = res.exec_time_ns
    _STATE["res"] = res
    out = np.concatenate([res.results[i]["out"] for i in range(NCORES)], 0)
    return out.astype(np.float32)


def last_exec_time_ns():
    return _STATE.get("exec_ns")
